# revision 1
# baseline (speedup 1.0000x reference)
"""Trainium2 Bass kernel for nn_CustomConv2d_32538672234916.

out[b,o,h,w] = K - sum_{ci,kh,kw} exp(x_patch)*exp(w) + bias[o],  K = Cin*kh*kw = 576
i.e. out = (K + bias) - conv2d(exp(x) [1-padded], exp(weight), stride 1)

Sharding: data-parallel over batch B=16 across 8 cores (2 images/core),
weights/bias replicated.

Per-core formulation (fp8 DoubleRow): SBUF holds exp(x) as fp8e4 (e4m3) on
128 partitions: partitions 0-63 = padded exp(x), partitions 64-127 = same
shifted down one padded row.  Each output strip of 7 rows is computed with
THREE fp8 DoubleRow matmuls accumulating in one PSUM bank.  The moving AP
must be exactly [partitions, 2 k-tiles, N] (3 dims), so a strip's free dim
is the CONTIGUOUS 7*66=462-element window of the padded image starting at
(row, col) = (h0+kh, kw); output columns c in [0,64) then map to real
output pixels with no row-wrap (kw+c <= 65), and c in {64,65} are junk
columns the drain skips.  Each DoubleRow matmul contracts 2 k-tiles of 128
partitions at 0.5 cycles/output-row, so the 9-tap x 64-Cin conv costs
3*231 cycles per 462-px strip -- 4x less PE time than the bf16 scheme:
  m1: k-tiles {taps (0,0)+(1,0)} and {taps (0,1)+(1,1)}   (ap i-stride 1)
  m2: k-tiles {taps (0,2)+(1,2)} and {tap (2,2), upper half zero-weighted}
      (ap i-stride 2*66)
  m3: k-tiles {tap (2,0)} and {tap (2,1)}, upper half zero-weighted
      (ap i-stride 1)
Weights are pre-negated and exp'd on host so PSUM = -conv; the drain adds
(K+bias[o]) per partition and converts to bf16 (split DVE/ACT to balance
engine time).  Output DRAM tensor is bf16 (halves the out-DMA bytes); host
upcasts to f32.
"""
import sys
sys.path.insert(0, '/opt/trn_rl_repo')
import numpy as np

B, CIN, H, W = 16, 64, 64, 64
COUT = 128
NCORES = 8
BL = B // NCORES          # images per core
PAD_W = W + 2             # 66
PAD_TOT = PAD_W * (H + 2) # 66*66 = 4356
KSUM = float(CIN * 9)     # 576
NROW = 7                  # rows per strip (7*66 = 462 <= 512 psum bank f32)
STRIPS = [(r, min(NROW, H - r)) for r in range(0, H, NROW)]  # 9x7 + 1x1
ET_PAD = 4                # bottom strip's last k-tile over-reads 2 elements

_CACHE = {}

# drain engine per global strip index d (20 per 2-image rep): GPSIMD cannot
# access PSUM on TRN2, so drains go to DVE (most) and ACT (a few); ACT also
# runs the exp activations so it gets the smaller share.
def _drain_on_act(d):
    return d in (4, 9, 14, 19)


def _build(reps=1, loop_n=None, out_dt="bf16", drain="mixed", debug=None):
    from concourse import bacc, mybir
    from concourse.tile import TileContext
    drain_mode = drain
    dbg = debug or ""

    f32 = mybir.dt.float32
    fp8 = mybir.dt.float8e4
    odt = {"bf16": mybir.dt.bfloat16, "f32": f32}[out_dt]
    Exp = mybir.ActivationFunctionType.Exp
    DR = mybir.MatmulPerfMode.DoubleRow

    nc = bacc.Bacc("TRN2", target_bir_lowering=False, debug=False)
    x_d = nc.dram_tensor("x", [BL, CIN, H, W], f32, kind="ExternalInput")
    wdbl_d = nc.dram_tensor("wdbl", [128, 3 * 2 * COUT], fp8, kind="ExternalInput")
    bvec_d = nc.dram_tensor("bvec", [COUT, 1], f32, kind="ExternalInput")
    out_d = nc.dram_tensor("out", [BL, COUT, H, W], odt, kind="ExternalOutput")
    x_ap = x_d.ap()
    out_ap = out_d.ap()

    HH = H // 2

    with TileContext(nc) as tc:
        with tc.tile_pool(name="consts", bufs=1) as consts, \
             tc.tile_pool(name="xp", bufs=2) as xp, \
             tc.tile_pool(name="ep", bufs=2) as ep, \
             tc.tile_pool(name="rp", bufs=2) as rp, \
             tc.tile_pool(name="pp", bufs=1, space="PSUM") as pp:
            wdbl_t = consts.tile([128, 3 * 2 * COUT], fp8)
            bv_t = consts.tile([COUT, 1], f32)
            nc.sync.dma_start(wdbl_t[:], wdbl_d.ap())
            nc.sync.dma_start(bv_t[:], bvec_d.ap())
            wv = wdbl_t.rearrange("p (k i o) -> p k i o", k=3, i=2)

            # exp(x) tiles: one per buffer slot; pad cells (exp(0-pad)=1.0)
            # are written once here and persist (reps rewrite interiors only).
            ET_SZ = PAD_TOT + ET_PAD
            ets = []
            for s in range(2):
                et = ep.tile([128, ET_SZ], fp8, tag="et", name=f"et_{s}")
                e3 = et[:, 0:PAD_TOT].rearrange("p (h w) -> p h w", w=PAD_W)
                nc.vector.memset(e3[0:64, 0, :], 1.0)            # top pad row
                nc.vector.memset(e3[0:64, H + 1, :], 1.0)        # bottom pad row
                nc.vector.memset(e3[0:64, 1:H + 1, 0], 1.0)      # left pad col
                nc.vector.memset(e3[0:64, 1:H + 1, W + 1], 1.0)  # right pad col
                # upper half: rows HH..H-1 col pads are written directly (not
                # via the dup DMA), and rows H..H+1 plus the 4-element tail
                # are read (zero-weighted / junk cols) -- must be finite.
                nc.vector.memset(e3[64:128, HH:H, 0], 1.0)
                nc.vector.memset(e3[64:128, HH:H, W + 1], 1.0)
                nc.vector.memset(e3[64:128, H:H + 2, :], 1.0)
                nc.vector.memset(et[:, PAD_TOT:ET_SZ], 1.0)
                ets.append(et)

            def mk_rhs(et, h0, istride, off, n):
                ap = et[:].copy()
                ap.ap = mybir.VecI64Pair(
                    [[ET_SZ, 128], [istride, 2], [1, n * PAD_W]])
                ap.offset = h0 * PAD_W + off
                return ap

            def body(i0, nrep):
                # one rep = one full per-core call = BL images
                for i in range(i0, i0 + nrep * BL):
                    b = i % BL
                    # x spread over all 128 partitions:
                    #   partition ci    <- x[b, ci, 0:32, :]
                    #   partition 64+ci <- x[b, ci, 32:64, :]
                    xt = xp.tile([128, HH * W], f32, tag="xt", name=f"xt_{i}")
                    nc.sync.dma_start(
                        xt[:], x_ap[b].rearrange("c (s h) w -> s c (h w)", s=2))
                    xt3 = xt.rearrange("p (h w) -> p h w", w=W)
                    et = ets[i % 2]
                    et3 = et[:, 0:PAD_TOT].rearrange("p (h w) -> p h w", w=PAD_W)
                    # half0 padded rows 1..32 = exp(x rows 0..31)
                    nc.scalar.activation(et3[0:CIN, 1:HH + 1, 1:W + 1],
                                         xt3[0:64], Exp)
                    # half1 rows 0..31 <- half0 rows 1..32 (incl col pads)
                    nc.sync.dma_start(et[64:128, 0:HH * PAD_W],
                                      et[0:64, PAD_W:(HH + 1) * PAD_W])
                    # half1 rows 32..63 = exp(x rows 32..63)
                    nc.scalar.activation(et3[64:128, HH:H, 1:W + 1],
                                         xt3[64:128], Exp)
                    # half0 rows 33..64 <- half1 rows 32..63 (incl col pads)
                    nc.sync.dma_start(
                        et[0:64, (HH + 1) * PAD_W:(H + 1) * PAD_W],
                        et[64:128, HH * PAD_W:H * PAD_W])

                    res = rp.tile([COUT, H * W], odt, tag="res",
                                  name=f"res_{i}")
                    r3 = res.rearrange("p (h w) -> p h w", w=W)

                    def drain(t):
                        if "nodrain" in dbg:
                            return
                        h0, n = STRIPS[t]
                        pv = pts[t].rearrange("p (r c) -> p r c", c=PAD_W)
                        d = (i % 2) * len(STRIPS) + t
                        if drain_mode == "mixed" and _drain_on_act(d):
                            nc.scalar.activation(
                                r3[:, h0:h0 + n, :], pv[:, 0:n, 0:W],
                                mybir.ActivationFunctionType.Identity,
                                bias=bv_t[:], scale=1.0)
                        else:
                            nc.vector.tensor_scalar(
                                r3[:, h0:h0 + n, :], pv[:, 0:n, 0:W],
                                bv_t[:], None, mybir.AluOpType.add)

                    strips = STRIPS[:9] if "notail" in dbg else STRIPS
                    if "nomm" in dbg:
                        nc.vector.memset(res[:], 0.0)
                        nc.sync.dma_start(
                            out_ap[b].rearrange("o h w -> o (h w)"), res[:])
                        continue
                    # tap-outer order within groups of 5 strips: weights stay
                    # loaded across each strip wave, <= 5 PSUM banks live.
                    pts = {}
                    for g0 in range(0, len(strips), 5):
                        grp = range(g0, min(g0 + 5, len(strips)))
                        # NOTE: DoubleRow k-pair strides must be EVEN -- odd
                        # i-strides (1, 133) crash the exec unit
                        # (NRT_EXEC_UNIT_UNRECOVERABLE); 2/130/132 are fine.
                        for t in grp:
                            h0, n = STRIPS[t]
                            # fixed-size tiles so psum tag reuse is uniform;
                            # the tail strip uses the first 66 elements.
                            pts[t] = pp.tile([COUT, NROW * PAD_W], f32,
                                             tag=f"pt{t % 8}", bufs=1,
                                             name=f"pt_{i}_{t}")
                            nc.tensor.matmul(pts[t][:, 0:n * PAD_W], wv[:, 0],
                                             mk_rhs(et, h0, 2, 0, n),
                                             start=True, stop=False, perf_mode=DR)
                        for t in grp:
                            h0, n = STRIPS[t]
                            nc.tensor.matmul(pts[t][:, 0:n * PAD_W], wv[:, 1],
                                             mk_rhs(et, h0, 2 * PAD_W, 1, n),
                                             start=False, stop=False, perf_mode=DR)
                        for t in grp:
                            h0, n = STRIPS[t]
                            nc.tensor.matmul(pts[t][:, 0:n * PAD_W], wv[:, 2],
                                             mk_rhs(et, h0 + 2, 2, 0, n),
                                             start=False, stop=True, perf_mode=DR)
                            drain(t)
                    nc.sync.dma_start(
                        out_ap[b].rearrange("o h w -> o (h w)"), res[:])

            if loop_n is None:
                body(0, reps)
            else:
                with tc.For_i(0, loop_n, 1):
                    body(0, reps)
    nc.compile()
    return nc


def _prep_inputs(weight, bias):
    import ml_dtypes
    ew = np.exp(weight.astype(np.float32))      # [COUT, CIN, 3, 3]
    neg = -ew
    # k-tile offsets rel. h0*66: T(kw)=+kw (kh 0/1 via halves), S(kw)=+132+kw
    # (tap (2,kw), lower half only).  Even-stride pairings:
    #   m1 = {T(0), T(2)}  stride 2;  m2 = {T(1), S(1)}  stride 132;
    #   m3 = {S(0), S(2)}  stride 2.
    wdbl = np.zeros((128, 3, 2, COUT), np.float32)
    for ci in range(CIN):
        wdbl[ci, 0, 0, :] = neg[:, ci, 0, 0]        # m1 i=0: T(0)
        wdbl[64 + ci, 0, 0, :] = neg[:, ci, 1, 0]
        wdbl[ci, 0, 1, :] = neg[:, ci, 0, 2]        # m1 i=1: T(2)
        wdbl[64 + ci, 0, 1, :] = neg[:, ci, 1, 2]
        wdbl[ci, 1, 0, :] = neg[:, ci, 0, 1]        # m2 i=0: T(1)
        wdbl[64 + ci, 1, 0, :] = neg[:, ci, 1, 1]
        wdbl[ci, 1, 1, :] = neg[:, ci, 2, 1]        # m2 i=1: S(1)
        wdbl[ci, 2, 0, :] = neg[:, ci, 2, 0]        # m3 i=0: S(0)
        wdbl[ci, 2, 1, :] = neg[:, ci, 2, 2]        # m3 i=1: S(2)
    wdbl8 = wdbl.reshape(128, 3 * 2 * COUT).astype(ml_dtypes.float8_e4m3)
    bvec = (KSUM + bias.astype(np.float32)).reshape(COUT, 1)
    return wdbl8, bvec


OUT_DT = "bf16"
DRAIN = "mixed"


def kernel(x, weight, bias):
    from concourse import bass_utils

    x = np.ascontiguousarray(np.asarray(x, dtype=np.float32))
    weight = np.asarray(weight, dtype=np.float32)
    bias = np.asarray(bias, dtype=np.float32)

    if "nc" not in _CACHE:
        _CACHE["nc"] = _build(out_dt=OUT_DT, drain=DRAIN)
    nc = _CACHE["nc"]

    wdbl, bvec = _prep_inputs(weight, bias)
    in_maps = [
        {"x": x[c * BL:(c + 1) * BL], "wdbl": wdbl, "bvec": bvec}
        for c in range(NCORES)
    ]
    res = bass_utils.run_bass_kernel_spmd(nc, in_maps, core_ids=list(range(NCORES)))
    return np.concatenate([r["out"] for r in res.results], axis=0).astype(np.float32)

